# revision 18
# baseline (speedup 1.0000x reference)
"""ARMA GNN (3 layers, N=50000 nodes, E=800000 edges, F=256) on 8 TRN2 NeuronCores.

Strategy:
  - Shard nodes across 8 cores (6250 each); partition edges by destination owner
    so the segment-sum is local to each core.
  - All graph structure (edge lists, GCN norm) is known when the kernel is built,
    so the host precomputes: per-(src-half, dst-block) edge runs, int16 gather
    indices, and dense 128x128 "S matrices" (S[e, d] = norm_e one-hot on the dst
    column).  On device the whole message-passing step is:
        gather h[src] rows (SWDGE dma_gather)  ->  PSUM += S_chunk^T @ G_chunk
    i.e. gather + scale + segment-sum fused into TensorEngine matmuls.
  - The Q7 gather-descriptor generator costs ~3us per call regardless of size,
    so gather calls are packed to exactly 1024 indices, spanning dst-block
    boundaries (a straddling chunk simply feeds two matmuls with complementary
    zero columns).  Per-dst-block PSUM accumulators are spilled to SBUF between
    the two src-half passes and reloaded with an identity matmul.
  - Per layer: h = x @ Wi in bf16, AllGather'd in two chunks (src-half A then B,
    so half-A gathers overlap half-B's collective); message matmuls + x @ Wr
    accumulate in PSUM; transposed epilogue fuses ReLU+bias on the Act engine.
    x lives feature-major (xT) in SBUF between layers; host transposes output.
"""

import numpy as np
import ml_dtypes

import concourse.bass as bass
import concourse.bacc as bacc
import concourse.mybir as mybir
import concourse.tile as tile
from concourse.bass_utils import run_bass_kernel_spmd
from concourse.masks import make_identity

BF16 = ml_dtypes.bfloat16

# Problem constants (hardcoded per harness contract).
N = 50000
E = 800000
F = 256
L = 3
C = 8                     # cores
NL = N // C               # nodes per core = 6250
NB = (NL + 127) // 128    # dst blocks per core = 49
_a = max(128, (NL // 3) // 128 * 128)
SPLITS = [_a, _a, NL - 2 * _a]    # local rows per src-split = [2048, 2048, 2154]
SOFF = [0, _a, 2 * _a]
NSP = 3
TBL = [C * sp for sp in SPLITS]   # gather tables — all < int16 max
NPAD = NB * 128           # padded local node count = 6272
GRING = 64                # G ring slots (chunks)


def _blkw(i):
    return NL - i * 128 if i == NB - 1 else 128


def _preprocess(x, edge_index, edge_attr, W_init, W_root, bias):
    """Host-side graph preprocessing. Returns (meta, per-core input maps)."""
    x = np.asarray(x, np.float32)
    ei = np.asarray(edge_index, np.int64)
    w = np.asarray(edge_attr, np.float32)
    W_init = np.asarray(W_init, np.float32)
    W_root = np.asarray(W_root, np.float32)
    bias = np.asarray(bias, np.float32)
    src, dst = ei[0], ei[1]

    deg = np.bincount(dst, weights=w.astype(np.float64), minlength=N).astype(np.float32)
    with np.errstate(divide="ignore"):
        dinv = np.where(deg > 0, 1.0 / np.sqrt(deg), 0.0).astype(np.float32)
    norm = (dinv[src] * w * dinv[dst]).astype(np.float32)

    core = dst // NL
    dloc = dst % NL
    db = dloc // 128
    dcol = dloc % 128
    sowner = src // NL
    sloc = src % NL
    half = np.digitize(sloc, [SOFF[1], SOFF[2]])   # split index 0..2
    spl = np.array(SPLITS)[half]
    sof = np.array(SOFF)[half]
    tbl = sowner * spl + (sloc - sof)

    # sort edges by (core, half, db, tbl)
    order = np.lexsort((tbl, db, half, core))
    g_core, g_half, g_db = core[order], half[order], db[order]
    g_tbl, g_norm, g_dcol = tbl[order], norm[order], dcol[order]

    # per-(core, half, db) counts -> unified run lengths (max over cores, SPMD)
    cnt = np.zeros((C, NSP, NB), np.int64)
    np.add.at(cnt, (g_core, g_half, g_db), 1)
    Lhb = cnt.max(axis=0)                      # [NSP, NB]

    run_key = (g_core * NSP + g_half) * NB + g_db
    starts = np.searchsorted(run_key, np.arange(C * NSP * NB))
    ends = np.append(starts[1:], len(run_key))

    # unified layout: per half, concatenated padded (h, db) runs; each half's
    # total padded up to a chunk (128) multiple
    off_hb = np.zeros((NSP, NB), np.int64)
    POS = [0] * NSP
    for h in range(NSP):
        p = 0
        for b in range(NB):
            off_hb[h, b] = p
            p += int(Lhb[h, b])
        POS[h] = -(-p // 128) * 128
    NCHUNK = [p // 128 for p in POS]

    # pieces: (chunk, db, s0, s1) — intersection of a 128-chunk with a run.
    # Emitted in position order (runs are disjoint ascending intervals).
    pieces = [[] for _ in range(NSP)]
    for h in range(NSP):
        for b in range(NB):
            lo, hi = int(off_hb[h, b]), int(off_hb[h, b] + Lhb[h, b])
            if lo == hi:
                continue
            for cc in range(lo // 128, (hi - 1) // 128 + 1):
                s0 = max(lo, cc * 128) - cc * 128
                s1 = min(hi, (cc + 1) * 128) - cc * 128
                pieces[h].append((cc, b, s0, s1))
        # position-ordered check (required for contiguous S streaming)
        ppos = [cc * 128 + s0 for (cc, b, s0, s1) in pieces[h]]
        assert all(ppos[i] < ppos[i + 1] for i in range(len(ppos) - 1))
    NPIECE = [len(p) for p in pieces]
    SW = sum(NPIECE) * 128
    WT = sum(POS) // 16

    # gather calls per split: groups of <=8 chunks
    calls = [[] for _ in range(NSP)]
    for h in range(NSP):
        cc = 0
        while cc < NCHUNK[h]:
            n = min(8, NCHUNK[h] - cc)
            calls[h].append((cc, n))
            cc += n
    # max pieces per call (St tile sizing)
    maxpc = 1
    for h in range(NSP):
        cp = {}
        for pi, (cc, b, s0, s1) in enumerate(pieces[h]):
            cp.setdefault(cc // 8, []).append(pi)
        maxpc = max([maxpc] + [len(v) for v in cp.values()])

    in_maps = []
    for ci in range(C):
        idx_all = np.zeros((128, WT), np.int16)
        s_all = np.zeros((128, SW), BF16)
        for h in range(NSP):
            base = sum(POS[:h]) // 16
            for b in range(NB):
                k = (ci * NSP + h) * NB + b
                a, e = starts[k], ends[k]
                ne = e - a
                if ne == 0:
                    continue
                j = int(off_hb[h, b]) + np.arange(ne)
                t16 = g_tbl[a:e].astype(np.int16)
                cols = base + j // 16
                rows = j % 16
                for grp in range(8):
                    idx_all[grp * 16 + rows, cols] = t16
        poff = 0
        for h in range(NSP):
            for (cc, b, s0, s1) in pieces[h]:
                k = (ci * NSP + h) * NB + b
                a, e = starts[k], ends[k]
                ne = e - a
                glo = cc * 128 + s0 - int(off_hb[h, b])
                ghi = cc * 128 + s1 - int(off_hb[h, b])
                lo, hi = max(0, glo), min(ne, ghi)
                if hi > lo:
                    jj = np.arange(lo, hi)
                    prow = (int(off_hb[h, b]) + jj) % 128
                    s_all[prow, poff + g_dcol[a + lo:a + hi]] = \
                        g_norm[a + lo:a + hi].astype(BF16)
                poff += 128
        xT = np.zeros((256, NPAD), BF16)
        xT[:, :NL] = x[ci * NL:(ci + 1) * NL].T.astype(BF16)
        in_maps.append(dict(
            xT=xT,
            idx_all=idx_all,
            s_all=s_all,
            wi=W_init.reshape(L, 2, 128, F).astype(BF16),
            wr=W_root.reshape(L, 2, 128, F).astype(BF16),
            bias_c=np.ascontiguousarray(
                bias.reshape(L * 2, 128).T.astype(np.float32)),  # [128, L*2]
        ))

    meta = dict(pieces=pieces, calls=calls, NCHUNK=NCHUNK, NPIECE=NPIECE,
                POS=POS, WT=WT, SW=SW, maxpc=maxpc)
    return meta, in_maps


def _nb_split(nb):
    c0 = nb * 128
    for sp_ in range(NSP - 1, -1, -1):
        if c0 >= SOFF[sp_]:
            return sp_


def _h_block(nc, psp, wp, l, nb, xsrc, wi_sb, hb):
    """Emit h = x @ Wi for one node block of layer l into its bounce buffer."""
    bf = mybir.dt.bfloat16
    f32 = mybir.dt.float32
    w = _blkw(nb)
    c0 = nb * 128
    ph = psp.tile([128, F], f32, tag="ph", bufs=3, name=f"ph{l}_{nb}")
    for g in range(2):
        nc.tensor.matmul(out=ph[:w, :], lhsT=xsrc[g][:, c0:c0 + w],
                         rhs=wi_sb[l][g][:], start=(g == 0), stop=(g == 1))
    hsb = wp.tile([128, F], bf, tag="hsb", bufs=6, name=f"hsb{l}_{nb}")
    nc.vector.tensor_copy(hsb[:w, :], ph[:w, :])
    sp_ = _nb_split(nb)
    r0 = c0 - SOFF[sp_]
    nc.sync.dma_start(out=hb[sp_][r0:r0 + w, :], in_=hsb[:w, :])


def _epilogue(nc, psp, wp, t, b, l, ident, bias_sb, xw_, outT, nxt):
    bf = mybir.dt.bfloat16
    f32 = mybir.dt.float32
    wd = _blkw(b)
    c0 = b * 128
    cp = wp.tile([128, F], bf, tag="cp", bufs=3, name=f"cp{l}_{b}")
    nc.vector.tensor_copy(cp[:], t[:])
    for g in range(2):
        pt = psp.tile([128, 128], bf, tag="pt", bufs=2, name=f"pt{l}_{b}_{g}")
        nc.tensor.transpose(out=pt[:], in_=cp[:, g * 128:(g + 1) * 128],
                            identity=ident[:])
        bcol = bias_sb[l * 2 + g][:]
        if l < L - 1:
            nc.scalar.activation(
                out=xw_[g][:, c0:c0 + wd], in_=pt[:, :wd],
                func=mybir.ActivationFunctionType.Relu, bias=bcol)
        else:
            ot = wp.tile([128, 128], f32, tag="ot", bufs=2, name=f"ot{l}_{b}_{g}")
            nc.scalar.activation(
                out=ot[:, :wd], in_=pt[:, :wd],
                func=mybir.ActivationFunctionType.Relu, bias=bcol)
            nc.sync.dma_start(out=outT[g][:, c0:c0 + wd], in_=ot[:, :wd])
    if l < L - 1:
        # queue next layer's h for this block; emitted with a lag so the PE
        # doesn't stall waiting on this epilogue's Act write
        nxt["pend"].append(b)
        _flush_h(nc, psp, wp, l, xw_, nxt, lag=4)


def _flush_h(nc, psp, wp, l, xw_, nxt, lag):
    while len(nxt["pend"]) > lag:
        b = nxt["pend"].pop(0)
        _h_block(nc, psp, wp, l + 1, b, xw_, nxt["wi_sb"], nxt["hb"])
        sp_ = _nb_split(b)
        nxt["rem"][sp_] -= 1
        if nxt["rem"][sp_] == 0:
            nc.gpsimd.collective_compute(
                "AllGather", mybir.AluOpType.bypass,
                replica_groups=nxt["groups"], ins=[nxt["hb"][sp_][:]],
                outs=[nxt["hg"][sp_][:]])


def _build(meta):
    pieces, calls = meta["pieces"], meta["calls"]
    POS, WT, SW = meta["POS"], meta["WT"], meta["SW"]
    NPIECE, maxpc = meta["NPIECE"], meta["maxpc"]
    bf = mybir.dt.bfloat16
    f32 = mybir.dt.float32

    nc = bacc.Bacc("TRN2", target_bir_lowering=False, debug=False, num_devices=C,
                   num_swdge_queues=4, dynamic_dma_scratch_size=32768)
    xT_p = nc.dram_tensor("xT", [256, NPAD], bf, kind="ExternalInput")
    idx_p = nc.dram_tensor("idx_all", [128, WT], mybir.dt.int16, kind="ExternalInput")
    s_p = nc.dram_tensor("s_all", [128, SW], bf, kind="ExternalInput")
    wi_p = nc.dram_tensor("wi", [L, 2, 128, F], bf, kind="ExternalInput")
    wr_p = nc.dram_tensor("wr", [L, 2, 128, F], bf, kind="ExternalInput")
    bias_p = nc.dram_tensor("bias_c", [128, L * 2], f32, kind="ExternalInput")
    outT = [nc.dram_tensor(f"outT{g}", [128, NL], f32, kind="ExternalOutput")
            for g in range(2)]

    groups = [list(range(C))]
    # chunk -> [(local_pi, cc, b, s0, s1)] and db -> last local_pi, per half
    chunk_pieces = [{} for _ in range(NSP)]
    db_last = [[None] * NB for _ in range(NSP)]
    db_any = [[False] * NB for _ in range(NSP)]
    for h in range(NSP):
        for pi, (cc, b, s0, s1) in enumerate(pieces[h]):
            chunk_pieces[h].setdefault(cc, []).append((pi, cc, b))
            db_last[h][b] = pi
            db_any[h][b] = True

    with tile.TileContext(nc) as tc:
        with (
            tc.tile_pool(name="persist", bufs=1) as pp,
            tc.tile_pool(name="dram", bufs=2, space="DRAM") as dp,
            tc.tile_pool(name="psum", bufs=3, space="PSUM") as psp,
            tc.tile_pool(name="work", bufs=3) as wp,
        ):
            ident = pp.tile([128, 128], bf)
            make_identity(nc, ident[:])
            idx_sb = pp.tile([128, WT], mybir.dt.int16)
            nc.sync.dma_start(out=idx_sb[:], in_=idx_p[:])
            bias_sb = [pp.tile([128, 1], f32, name=f"bias{c}") for c in range(L * 2)]
            for c_ in range(L * 2):
                nc.sync.dma_start(out=bias_sb[c_][:], in_=bias_p[:, c_:c_ + 1])
            wi_sb = [[pp.tile([128, F], bf, name=f"wi{l}{g}") for g in range(2)]
                     for l in range(L)]
            wr_sb = [[pp.tile([128, F], bf, name=f"wr{l}{g}") for g in range(2)]
                     for l in range(L)]
            for l in range(L):
                for g in range(2):
                    nc.sync.dma_start(out=wi_sb[l][g][:], in_=wi_p[l, g])
                    nc.sync.dma_start(out=wr_sb[l][g][:], in_=wr_p[l, g])
            xa = [pp.tile([128, NPAD], bf, name=f"xa{g}") for g in range(2)]
            xb = [pp.tile([128, NPAD], bf, name=f"xb{g}") for g in range(2)]
            for g in range(2):
                nc.sync.dma_start(out=xa[g][:], in_=xT_p[g * 128:(g + 1) * 128, :])
                if NPAD > NL:
                    nc.gpsimd.memset(xb[g][:, NL:], 0.0)
            acc_sb = pp.tile([128, NB * F], bf)   # pass A -> pass B spill

            # per-layer bounce/gather-table tiles (bufs=2 ping-pong)
            hb = []
            hg = []
            for l in range(L):
                hb.append(tuple(dp.tile([SPLITS[sp_], F], bf, tag=f"hb{sp_}",
                                        name=f"hb{sp_}_{l}")
                                for sp_ in range(NSP)))
                hg.append(tuple(dp.tile([TBL[sp_], F], bf, addr_space="Shared",
                                        tag=f"hg{sp_}", name=f"hg{sp_}_{l}")
                                for sp_ in range(NSP)))

            # prologue: layer 0 h-phase + collectives
            nbounds = [SOFF[1] // 128, SOFF[2] // 128, NB]
            for nb in range(NB):
                _h_block(nc, psp, wp, 0, nb, xa, wi_sb, hb[0])
                for sp_ in range(NSP):
                    if nb == nbounds[sp_] - 1:
                        nc.gpsimd.collective_compute(
                            "AllGather", mybir.AluOpType.bypass,
                            replica_groups=groups, ins=[hb[0][sp_][:]],
                            outs=[hg[0][sp_][:]])

            qn = [0]
            for l in range(L):
                xr_ = xa if l % 2 == 0 else xb
                xw_ = xb if l % 2 == 0 else xa
                if l < L - 1:
                    nxt = dict(wi_sb=wi_sb, hb=hb[l + 1], hg=hg[l + 1],
                               rem=[SOFF[1] // 128, (SOFF[2] - SOFF[1]) // 128,
                                    NB - SOFF[2] // 128],
                               groups=groups, pend=[])
                else:
                    nxt = None

                # ---- message passing: pass A then pass B ----
                Gr = pp.tile([128, GRING, F], bf, name=f"Gr{l}", tag="Gr")
                pa = {}
                for h in range(NSP):
                    hgx = hg[l][h]
                    ibase = sum(POS[:h]) // 16
                    pbase = sum(NPIECE[:h])
                    for (clo, ncnk) in calls[h]:
                        slot0 = clo % GRING
                        nidx = ncnk * 128
                        nc.gpsimd.dma_gather(
                            out_ap=Gr[:, slot0:slot0 + ncnk, :], in_ap=hgx[:],
                            idxs_ap=idx_sb[:, ibase + clo * 8:
                                           ibase + (clo + ncnk) * 8],
                            num_idxs=nidx, num_idxs_reg=nidx,
                            elem_size=F, queue_num=qn[0] % 4)
                        qn[0] += 1
                        plist = []
                        for cc in range(clo, clo + ncnk):
                            plist += chunk_pieces[h].get(cc, [])
                        if not plist:
                            continue
                        p0 = plist[0][0]
                        np_ = len(plist)
                        St = wp.tile([128, maxpc * 128], bf, tag="St", bufs=4,
                                     name=f"St{l}_{h}_{clo}")
                        nc.sync.dma_start(
                            out=St[:, :np_ * 128],
                            in_=s_p[:, (pbase + p0) * 128:(pbase + p0 + np_) * 128])
                        for k, (pi, cc, b) in enumerate(plist):
                            if b not in pa:
                                t = psp.tile([128, F], f32, tag="pa", bufs=2,
                                             name=f"pa{l}_{h}_{b}")
                                pa[b] = t
                                if any(db_any[hh][b] for hh in range(h)):
                                    nc.tensor.matmul(
                                        out=t[:], lhsT=ident[:],
                                        rhs=acc_sb[:, b * F:(b + 1) * F],
                                        start=True, stop=False)
                                else:
                                    for g in range(2):
                                        nc.tensor.matmul(
                                            out=t[:],
                                            lhsT=xr_[g][:, b * 128:b * 128 + 128],
                                            rhs=wr_sb[l][g][:],
                                            start=(g == 0), stop=False)
                            last_piece = (pi == db_last[h][b])
                            nc.tensor.matmul(
                                out=pa[b][:],
                                lhsT=St[:, k * 128:(k + 1) * 128],
                                rhs=Gr[:, slot0 + (cc - clo), :],
                                start=False, stop=last_piece)
                            if last_piece:
                                t = pa.pop(b)
                                if any(db_any[hh][b] for hh in range(h + 1, NSP)):
                                    nc.vector.tensor_copy(
                                        acc_sb[:, b * F:(b + 1) * F], t[:])
                                else:
                                    _epilogue(nc, psp, wp, t, b, l, ident,
                                              bias_sb, xw_, outT, nxt)
                assert not pa
                if nxt is not None:
                    _flush_h(nc, psp, wp, l, xw_, nxt, lag=0)
                # dbs with no edges at all (xr + bias + relu only)
                for b in range(NB):
                    if not any(db_any[hh][b] for hh in range(NSP)):
                        t = psp.tile([128, F], f32, tag="pa", bufs=2,
                                     name=f"paz{l}_{b}")
                        for g in range(2):
                            nc.tensor.matmul(
                                out=t[:], lhsT=xr_[g][:, b * 128:b * 128 + 128],
                                rhs=wr_sb[l][g][:], start=(g == 0), stop=(g == 1))
                        _epilogue(nc, psp, wp, t, b, l, ident, bias_sb,
                                  xw_, outT, nxt)
    nc.compile()
    return nc


_CACHE = {}


def kernel(**inputs):
    meta, in_maps = _preprocess(**inputs)
    key = (tuple(map(tuple, meta["calls"][0])), tuple(map(tuple, meta["calls"][1])),
           tuple(map(tuple, meta["pieces"][0])), tuple(map(tuple, meta["pieces"][1])))
    nc = _CACHE.get(key)
    if nc is None:
        nc = _build(meta)
        _CACHE[key] = nc
    res = run_bass_kernel_spmd(nc, in_maps, list(range(C)), trace=False)
    out = np.empty((N, F), np.float32)
    for ci in range(C):
        r = res.results[ci]
        xt = np.concatenate([r["outT0"], r["outT1"]], axis=0)  # [256, NL]
        out[ci * NL:(ci + 1) * NL] = xt.T
    return out


# revision 20
# speedup vs baseline: 1.0537x; 1.0537x over previous
"""ARMA GNN (3 layers, N=50000 nodes, E=800000 edges, F=256) on 8 TRN2 NeuronCores.

Strategy:
  - Shard nodes across 8 cores (6250 each); partition edges by destination owner
    so the segment-sum is local to each core.
  - All graph structure (edge lists, GCN norm) is known when the kernel is built,
    so the host precomputes: per-(src-half, dst-block) edge runs, int16 gather
    indices, and dense 128x128 "S matrices" (S[e, d] = norm_e one-hot on the dst
    column).  On device the whole message-passing step is:
        gather h[src] rows (SWDGE dma_gather)  ->  PSUM += S_chunk^T @ G_chunk
    i.e. gather + scale + segment-sum fused into TensorEngine matmuls.
  - The Q7 gather-descriptor generator costs ~3us per call regardless of size,
    so gather calls are packed to exactly 1024 indices, spanning dst-block
    boundaries (a straddling chunk simply feeds two matmuls with complementary
    zero columns).  Per-dst-block PSUM accumulators are spilled to SBUF between
    the two src-half passes and reloaded with an identity matmul.
  - Per layer: h = x @ Wi in bf16, AllGather'd in two chunks (src-half A then B,
    so half-A gathers overlap half-B's collective); message matmuls + x @ Wr
    accumulate in PSUM; transposed epilogue fuses ReLU+bias on the Act engine.
    x lives feature-major (xT) in SBUF between layers; host transposes output.
"""

import numpy as np
import ml_dtypes

import concourse.bass as bass
import concourse.bacc as bacc
import concourse.mybir as mybir
import concourse.tile as tile
from concourse.bass_utils import run_bass_kernel_spmd
from concourse.masks import make_identity

BF16 = ml_dtypes.bfloat16

# Problem constants (hardcoded per harness contract).
N = 50000
E = 800000
F = 256
L = 3
C = 8                     # cores
NL = N // C               # nodes per core = 6250
NB = (NL + 127) // 128    # dst blocks per core = 49
_a = max(128, (NL * 41 // 100) // 128 * 128)
SPLITS = [_a, NL - _a]            # local rows per src-split = [2560, 3690]
SOFF = [0, _a]
NSP = 2
TBL = [C * sp for sp in SPLITS]   # gather tables — all < int16 max
NPAD = NB * 128           # padded local node count = 6272
GRING = 64                # G ring slots (chunks)


def _blkw(i):
    return NL - i * 128 if i == NB - 1 else 128


def _preprocess(x, edge_index, edge_attr, W_init, W_root, bias):
    """Host-side graph preprocessing. Returns (meta, per-core input maps)."""
    x = np.asarray(x, np.float32)
    ei = np.asarray(edge_index, np.int64)
    w = np.asarray(edge_attr, np.float32)
    W_init = np.asarray(W_init, np.float32)
    W_root = np.asarray(W_root, np.float32)
    bias = np.asarray(bias, np.float32)
    src, dst = ei[0], ei[1]

    deg = np.bincount(dst, weights=w.astype(np.float64), minlength=N).astype(np.float32)
    with np.errstate(divide="ignore"):
        dinv = np.where(deg > 0, 1.0 / np.sqrt(deg), 0.0).astype(np.float32)
    norm = (dinv[src] * w * dinv[dst]).astype(np.float32)

    core = dst // NL
    dloc = dst % NL
    db = dloc // 128
    dcol = dloc % 128
    sowner = src // NL
    sloc = src % NL
    half = np.digitize(sloc, SOFF[1:])   # split index
    spl = np.array(SPLITS)[half]
    sof = np.array(SOFF)[half]
    tbl = sowner * spl + (sloc - sof)

    # sort edges by (core, half, db, tbl)
    order = np.lexsort((tbl, db, half, core))
    g_core, g_half, g_db = core[order], half[order], db[order]
    g_tbl, g_norm, g_dcol = tbl[order], norm[order], dcol[order]

    # per-(core, half, db) counts -> unified run lengths (max over cores, SPMD)
    cnt = np.zeros((C, NSP, NB), np.int64)
    np.add.at(cnt, (g_core, g_half, g_db), 1)
    Lhb = cnt.max(axis=0)                      # [NSP, NB]

    run_key = (g_core * NSP + g_half) * NB + g_db
    starts = np.searchsorted(run_key, np.arange(C * NSP * NB))
    ends = np.append(starts[1:], len(run_key))

    # unified layout: per half, concatenated padded (h, db) runs; each half's
    # total padded up to a chunk (128) multiple
    off_hb = np.zeros((NSP, NB), np.int64)
    POS = [0] * NSP
    for h in range(NSP):
        p = 0
        for b in range(NB):
            off_hb[h, b] = p
            p += int(Lhb[h, b])
        POS[h] = -(-p // 128) * 128
    NCHUNK = [p // 128 for p in POS]

    # pieces: (chunk, db, s0, s1) — intersection of a 128-chunk with a run.
    # Emitted in position order (runs are disjoint ascending intervals).
    pieces = [[] for _ in range(NSP)]
    for h in range(NSP):
        for b in range(NB):
            lo, hi = int(off_hb[h, b]), int(off_hb[h, b] + Lhb[h, b])
            if lo == hi:
                continue
            for cc in range(lo // 128, (hi - 1) // 128 + 1):
                s0 = max(lo, cc * 128) - cc * 128
                s1 = min(hi, (cc + 1) * 128) - cc * 128
                pieces[h].append((cc, b, s0, s1))
        # position-ordered check (required for contiguous S streaming)
        ppos = [cc * 128 + s0 for (cc, b, s0, s1) in pieces[h]]
        assert all(ppos[i] < ppos[i + 1] for i in range(len(ppos) - 1))
    NPIECE = [len(p) for p in pieces]
    SW = sum(NPIECE) * 128
    WT = sum(POS) // 16

    # gather calls per split: groups of <=8 chunks
    calls = [[] for _ in range(NSP)]
    for h in range(NSP):
        cc = 0
        while cc < NCHUNK[h]:
            n = min(8, NCHUNK[h] - cc)
            calls[h].append((cc, n))
            cc += n
    # max pieces per call (St tile sizing)
    maxpc = 1
    for h in range(NSP):
        cp = {}
        for pi, (cc, b, s0, s1) in enumerate(pieces[h]):
            cp.setdefault(cc // 8, []).append(pi)
        maxpc = max([maxpc] + [len(v) for v in cp.values()])

    in_maps = []
    for ci in range(C):
        idx_all = np.zeros((128, WT), np.int16)
        s_all = np.zeros((128, SW), BF16)
        for h in range(NSP):
            base = sum(POS[:h]) // 16
            for b in range(NB):
                k = (ci * NSP + h) * NB + b
                a, e = starts[k], ends[k]
                ne = e - a
                if ne == 0:
                    continue
                j = int(off_hb[h, b]) + np.arange(ne)
                t16 = g_tbl[a:e].astype(np.int16)
                cols = base + j // 16
                rows = j % 16
                for grp in range(8):
                    idx_all[grp * 16 + rows, cols] = t16
        poff = 0
        for h in range(NSP):
            for (cc, b, s0, s1) in pieces[h]:
                k = (ci * NSP + h) * NB + b
                a, e = starts[k], ends[k]
                ne = e - a
                glo = cc * 128 + s0 - int(off_hb[h, b])
                ghi = cc * 128 + s1 - int(off_hb[h, b])
                lo, hi = max(0, glo), min(ne, ghi)
                if hi > lo:
                    jj = np.arange(lo, hi)
                    prow = (int(off_hb[h, b]) + jj) % 128
                    s_all[prow, poff + g_dcol[a + lo:a + hi]] = \
                        g_norm[a + lo:a + hi].astype(BF16)
                poff += 128
        xT = np.zeros((256, NPAD), BF16)
        xT[:, :NL] = x[ci * NL:(ci + 1) * NL].T.astype(BF16)
        in_maps.append(dict(
            xT=xT,
            idx_all=idx_all,
            s_all=s_all,
            wi=W_init.reshape(L, 2, 128, F).astype(BF16),
            wr=W_root.reshape(L, 2, 128, F).astype(BF16),
            bias_c=np.ascontiguousarray(
                bias.reshape(L * 2, 128).T.astype(np.float32)),  # [128, L*2]
        ))

    meta = dict(pieces=pieces, calls=calls, NCHUNK=NCHUNK, NPIECE=NPIECE,
                POS=POS, WT=WT, SW=SW, maxpc=maxpc)
    return meta, in_maps


def _nb_split(nb):
    c0 = nb * 128
    for sp_ in range(NSP - 1, -1, -1):
        if c0 >= SOFF[sp_]:
            return sp_


def _h_block(nc, psp, wp, l, nb, xsrc, wi_sb, hb):
    """Emit h = x @ Wi for one node block of layer l into its bounce buffer."""
    bf = mybir.dt.bfloat16
    f32 = mybir.dt.float32
    w = _blkw(nb)
    c0 = nb * 128
    ph = psp.tile([128, F], f32, tag="ph", bufs=3, name=f"ph{l}_{nb}")
    for g in range(2):
        nc.tensor.matmul(out=ph[:w, :], lhsT=xsrc[g][:, c0:c0 + w],
                         rhs=wi_sb[l][g][:], start=(g == 0), stop=(g == 1))
    hsb = wp.tile([128, F], bf, tag="hsb", bufs=6, name=f"hsb{l}_{nb}")
    nc.vector.tensor_copy(hsb[:w, :], ph[:w, :])
    sp_ = _nb_split(nb)
    r0 = c0 - SOFF[sp_]
    nc.sync.dma_start(out=hb[sp_][r0:r0 + w, :], in_=hsb[:w, :])


def _epilogue(nc, psp, wp, t, b, l, ident, bias_sb, xw_, outT, nxt):
    bf = mybir.dt.bfloat16
    f32 = mybir.dt.float32
    wd = _blkw(b)
    c0 = b * 128
    cp = wp.tile([128, F], bf, tag="cp", bufs=3, name=f"cp{l}_{b}")
    nc.vector.tensor_copy(cp[:], t[:])
    for g in range(2):
        pt = psp.tile([128, 128], bf, tag="pt", bufs=2, name=f"pt{l}_{b}_{g}")
        nc.tensor.transpose(out=pt[:], in_=cp[:, g * 128:(g + 1) * 128],
                            identity=ident[:])
        bcol = bias_sb[l * 2 + g][:]
        if l < L - 1:
            nc.scalar.activation(
                out=xw_[g][:, c0:c0 + wd], in_=pt[:, :wd],
                func=mybir.ActivationFunctionType.Relu, bias=bcol)
        else:
            ot = wp.tile([128, 128], f32, tag="ot", bufs=2, name=f"ot{l}_{b}_{g}")
            nc.scalar.activation(
                out=ot[:, :wd], in_=pt[:, :wd],
                func=mybir.ActivationFunctionType.Relu, bias=bcol)
            nc.sync.dma_start(out=outT[g][:, c0:c0 + wd], in_=ot[:, :wd])
    if l < L - 1:
        # queue next layer's h for this block; emitted with a lag so the PE
        # doesn't stall waiting on this epilogue's Act write
        nxt["pend"].append(b)
        _flush_h(nc, psp, wp, l, xw_, nxt, lag=10**9)


def _flush_h(nc, psp, wp, l, xw_, nxt, lag):
    while len(nxt["pend"]) > lag:
        b = nxt["pend"].pop(0)
        _h_block(nc, psp, wp, l + 1, b, xw_, nxt["wi_sb"], nxt["hb"])
        sp_ = _nb_split(b)
        nxt["rem"][sp_] -= 1
        if nxt["rem"][sp_] == 0:
            nc.gpsimd.collective_compute(
                "AllGather", mybir.AluOpType.bypass,
                replica_groups=nxt["groups"], ins=[nxt["hb"][sp_][:]],
                outs=[nxt["hg"][sp_][:]])


def _build(meta):
    pieces, calls = meta["pieces"], meta["calls"]
    POS, WT, SW = meta["POS"], meta["WT"], meta["SW"]
    NPIECE, maxpc = meta["NPIECE"], meta["maxpc"]
    bf = mybir.dt.bfloat16
    f32 = mybir.dt.float32

    nc = bacc.Bacc("TRN2", target_bir_lowering=False, debug=False, num_devices=C,
                   num_swdge_queues=4, dynamic_dma_scratch_size=32768)
    xT_p = nc.dram_tensor("xT", [256, NPAD], bf, kind="ExternalInput")
    idx_p = nc.dram_tensor("idx_all", [128, WT], mybir.dt.int16, kind="ExternalInput")
    s_p = nc.dram_tensor("s_all", [128, SW], bf, kind="ExternalInput")
    wi_p = nc.dram_tensor("wi", [L, 2, 128, F], bf, kind="ExternalInput")
    wr_p = nc.dram_tensor("wr", [L, 2, 128, F], bf, kind="ExternalInput")
    bias_p = nc.dram_tensor("bias_c", [128, L * 2], f32, kind="ExternalInput")
    outT = [nc.dram_tensor(f"outT{g}", [128, NL], f32, kind="ExternalOutput")
            for g in range(2)]

    groups = [list(range(C))]
    # chunk -> [(local_pi, cc, b, s0, s1)] and db -> last local_pi, per half
    chunk_pieces = [{} for _ in range(NSP)]
    db_last = [[None] * NB for _ in range(NSP)]
    db_any = [[False] * NB for _ in range(NSP)]
    for h in range(NSP):
        for pi, (cc, b, s0, s1) in enumerate(pieces[h]):
            chunk_pieces[h].setdefault(cc, []).append((pi, cc, b))
            db_last[h][b] = pi
            db_any[h][b] = True

    with tile.TileContext(nc) as tc:
        with (
            tc.tile_pool(name="persist", bufs=1) as pp,
            tc.tile_pool(name="dram", bufs=2, space="DRAM") as dp,
            tc.tile_pool(name="psum", bufs=3, space="PSUM") as psp,
            tc.tile_pool(name="work", bufs=3) as wp,
        ):
            ident = pp.tile([128, 128], bf)
            make_identity(nc, ident[:])
            idx_sb = pp.tile([128, WT], mybir.dt.int16)
            nc.sync.dma_start(out=idx_sb[:], in_=idx_p[:])
            bias_sb = [pp.tile([128, 1], f32, name=f"bias{c}") for c in range(L * 2)]
            for c_ in range(L * 2):
                nc.sync.dma_start(out=bias_sb[c_][:], in_=bias_p[:, c_:c_ + 1])
            wi_sb = [[pp.tile([128, F], bf, name=f"wi{l}{g}") for g in range(2)]
                     for l in range(L)]
            wr_sb = [[pp.tile([128, F], bf, name=f"wr{l}{g}") for g in range(2)]
                     for l in range(L)]
            for l in range(L):
                for g in range(2):
                    nc.sync.dma_start(out=wi_sb[l][g][:], in_=wi_p[l, g])
                    nc.sync.dma_start(out=wr_sb[l][g][:], in_=wr_p[l, g])
            xa = [pp.tile([128, NPAD], bf, name=f"xa{g}") for g in range(2)]
            xb = [pp.tile([128, NPAD], bf, name=f"xb{g}") for g in range(2)]
            for g in range(2):
                nc.sync.dma_start(out=xa[g][:], in_=xT_p[g * 128:(g + 1) * 128, :])
                if NPAD > NL:
                    nc.gpsimd.memset(xb[g][:, NL:], 0.0)
            acc_sb = pp.tile([128, NB * F], bf)   # pass A -> pass B spill

            # per-layer bounce/gather-table tiles (bufs=2 ping-pong)
            hb = []
            hg = []
            for l in range(L):
                hb.append(tuple(dp.tile([SPLITS[sp_], F], bf, tag=f"hb{sp_}",
                                        name=f"hb{sp_}_{l}")
                                for sp_ in range(NSP)))
                hg.append(tuple(dp.tile([TBL[sp_], F], bf, addr_space="Shared",
                                        tag=f"hg{sp_}", name=f"hg{sp_}_{l}")
                                for sp_ in range(NSP)))

            # prologue: layer 0 h-phase + collectives
            nbounds = [SOFF[i] // 128 for i in range(1, NSP)] + [NB]
            for nb in range(NB):
                _h_block(nc, psp, wp, 0, nb, xa, wi_sb, hb[0])
                for sp_ in range(NSP):
                    if nb == nbounds[sp_] - 1:
                        nc.gpsimd.collective_compute(
                            "AllGather", mybir.AluOpType.bypass,
                            replica_groups=groups, ins=[hb[0][sp_][:]],
                            outs=[hg[0][sp_][:]])

            qn = [0]
            for l in range(L):
                xr_ = xa if l % 2 == 0 else xb
                xw_ = xb if l % 2 == 0 else xa
                if l < L - 1:
                    nxt = dict(wi_sb=wi_sb, hb=hb[l + 1], hg=hg[l + 1],
                               rem=[(SOFF + [NL])[i + 1] // 128 - SOFF[i] // 128
                                    if i < NSP - 1 else NB - SOFF[i] // 128
                                    for i in range(NSP)],
                               groups=groups, pend=[])
                else:
                    nxt = None

                # ---- message passing: pass A then pass B ----
                Gr = pp.tile([128, GRING, F], bf, name=f"Gr{l}", tag="Gr")
                pa = {}
                for h in range(NSP):
                    hgx = hg[l][h]
                    ibase = sum(POS[:h]) // 16
                    pbase = sum(NPIECE[:h])
                    for (clo, ncnk) in calls[h]:
                        slot0 = clo % GRING
                        nidx = ncnk * 128
                        nc.gpsimd.dma_gather(
                            out_ap=Gr[:, slot0:slot0 + ncnk, :], in_ap=hgx[:],
                            idxs_ap=idx_sb[:, ibase + clo * 8:
                                           ibase + (clo + ncnk) * 8],
                            num_idxs=nidx, num_idxs_reg=nidx,
                            elem_size=F, queue_num=qn[0] % 4)
                        qn[0] += 1
                        plist = []
                        for cc in range(clo, clo + ncnk):
                            plist += chunk_pieces[h].get(cc, [])
                        if not plist:
                            continue
                        p0 = plist[0][0]
                        np_ = len(plist)
                        St = wp.tile([128, maxpc * 128], bf, tag="St", bufs=4,
                                     name=f"St{l}_{h}_{clo}")
                        nc.sync.dma_start(
                            out=St[:, :np_ * 128],
                            in_=s_p[:, (pbase + p0) * 128:(pbase + p0 + np_) * 128])
                        for k, (pi, cc, b) in enumerate(plist):
                            if b not in pa:
                                t = psp.tile([128, F], f32, tag="pa", bufs=3,
                                             name=f"pa{l}_{h}_{b}")
                                pa[b] = t
                                if any(db_any[hh][b] for hh in range(h)):
                                    nc.tensor.matmul(
                                        out=t[:], lhsT=ident[:],
                                        rhs=acc_sb[:, b * F:(b + 1) * F],
                                        start=True, stop=False)
                                else:
                                    for g in range(2):
                                        nc.tensor.matmul(
                                            out=t[:],
                                            lhsT=xr_[g][:, b * 128:b * 128 + 128],
                                            rhs=wr_sb[l][g][:],
                                            start=(g == 0), stop=False)
                            last_piece = (pi == db_last[h][b])
                            nc.tensor.matmul(
                                out=pa[b][:],
                                lhsT=St[:, k * 128:(k + 1) * 128],
                                rhs=Gr[:, slot0 + (cc - clo), :],
                                start=False, stop=last_piece)
                            if last_piece:
                                t = pa.pop(b)
                                if any(db_any[hh][b] for hh in range(h + 1, NSP)):
                                    nc.vector.tensor_copy(
                                        acc_sb[:, b * F:(b + 1) * F], t[:])
                                else:
                                    _epilogue(nc, psp, wp, t, b, l, ident,
                                              bias_sb, xw_, outT, nxt)
                assert not pa
                if nxt is not None:
                    _flush_h(nc, psp, wp, l, xw_, nxt, lag=0)
                # dbs with no edges at all (xr + bias + relu only)
                for b in range(NB):
                    if not any(db_any[hh][b] for hh in range(NSP)):
                        t = psp.tile([128, F], f32, tag="pa", bufs=3,
                                     name=f"paz{l}_{b}")
                        for g in range(2):
                            nc.tensor.matmul(
                                out=t[:], lhsT=xr_[g][:, b * 128:b * 128 + 128],
                                rhs=wr_sb[l][g][:], start=(g == 0), stop=(g == 1))
                        _epilogue(nc, psp, wp, t, b, l, ident, bias_sb,
                                  xw_, outT, nxt)
    nc.compile()
    return nc


_CACHE = {}


def kernel(**inputs):
    meta, in_maps = _preprocess(**inputs)
    key = (tuple(map(tuple, meta["calls"][0])), tuple(map(tuple, meta["calls"][1])),
           tuple(map(tuple, meta["pieces"][0])), tuple(map(tuple, meta["pieces"][1])))
    nc = _CACHE.get(key)
    if nc is None:
        nc = _build(meta)
        _CACHE[key] = nc
    res = run_bass_kernel_spmd(nc, in_maps, list(range(C)), trace=False)
    out = np.empty((N, F), np.float32)
    for ci in range(C):
        r = res.results[ci]
        xt = np.concatenate([r["outT0"], r["outT1"]], axis=0)  # [256, NL]
        out[ci * NL:(ci + 1) * NL] = xt.T
    return out


# revision 21
# speedup vs baseline: 1.1159x; 1.0591x over previous
"""ARMA GNN (3 layers, N=50000 nodes, E=800000 edges, F=256) on 8 TRN2 NeuronCores.

Strategy:
  - Shard nodes across 8 cores (6250 each); partition edges by destination owner
    so the segment-sum is local to each core.
  - All graph structure (edge lists, GCN norm) is known when the kernel is built,
    so the host precomputes: per-(src-half, dst-block) edge runs, int16 gather
    indices, and dense 128x128 "S matrices" (S[e, d] = norm_e one-hot on the dst
    column).  On device the whole message-passing step is:
        gather h[src] rows (SWDGE dma_gather)  ->  PSUM += S_chunk^T @ G_chunk
    i.e. gather + scale + segment-sum fused into TensorEngine matmuls.
  - The Q7 gather-descriptor generator costs ~3us per call regardless of size,
    so gather calls are packed to exactly 1024 indices, spanning dst-block
    boundaries (a straddling chunk simply feeds two matmuls with complementary
    zero columns).  Per-dst-block PSUM accumulators are spilled to SBUF between
    the two src-half passes and reloaded with an identity matmul.
  - Per layer: h = x @ Wi in bf16, AllGather'd in two chunks (src-half A then B,
    so half-A gathers overlap half-B's collective); message matmuls + x @ Wr
    accumulate in PSUM; transposed epilogue fuses ReLU+bias on the Act engine.
    x lives feature-major (xT) in SBUF between layers; host transposes output.
"""

import numpy as np
import ml_dtypes

import concourse.bass as bass
import concourse.bacc as bacc
import concourse.mybir as mybir
import concourse.tile as tile
from concourse.bass_utils import run_bass_kernel_spmd
from concourse.masks import make_identity

BF16 = ml_dtypes.bfloat16

# Problem constants (hardcoded per harness contract).
N = 50000
E = 800000
F = 256
L = 3
C = 8                     # cores
NL = N // C               # nodes per core = 6250
NB = (NL + 127) // 128    # dst blocks per core = 49
_a = max(128, (NL * 41 // 100) // 128 * 128)
SPLITS = [_a, NL - _a]            # local rows per src-split = [2560, 3690]
SOFF = [0, _a]
NSP = 2
TBL = [C * sp for sp in SPLITS]   # gather tables — all < int16 max
NPAD = NB * 128           # padded local node count = 6272
GRING = 64                # G ring slots (chunks)


def _blkw(i):
    return NL - i * 128 if i == NB - 1 else 128


def _preprocess(x, edge_index, edge_attr, W_init, W_root, bias):
    """Host-side graph preprocessing. Returns (meta, per-core input maps)."""
    x = np.asarray(x, np.float32)
    ei = np.asarray(edge_index, np.int64)
    w = np.asarray(edge_attr, np.float32)
    W_init = np.asarray(W_init, np.float32)
    W_root = np.asarray(W_root, np.float32)
    bias = np.asarray(bias, np.float32)
    src, dst = ei[0], ei[1]

    deg = np.bincount(dst, weights=w.astype(np.float64), minlength=N).astype(np.float32)
    with np.errstate(divide="ignore"):
        dinv = np.where(deg > 0, 1.0 / np.sqrt(deg), 0.0).astype(np.float32)
    norm = (dinv[src] * w * dinv[dst]).astype(np.float32)

    core = dst // NL
    dloc = dst % NL
    db = dloc // 128
    dcol = dloc % 128
    sowner = src // NL
    sloc = src % NL
    half = np.digitize(sloc, SOFF[1:])   # split index
    spl = np.array(SPLITS)[half]
    sof = np.array(SOFF)[half]
    tbl = sowner * spl + (sloc - sof)

    # sort edges by (core, half, db, tbl)
    order = np.lexsort((tbl, db, half, core))
    g_core, g_half, g_db = core[order], half[order], db[order]
    g_tbl, g_norm, g_dcol = tbl[order], norm[order], dcol[order]

    # per-(core, half, db) counts -> unified run lengths (max over cores, SPMD)
    cnt = np.zeros((C, NSP, NB), np.int64)
    np.add.at(cnt, (g_core, g_half, g_db), 1)
    Lhb = cnt.max(axis=0)                      # [NSP, NB]

    run_key = (g_core * NSP + g_half) * NB + g_db
    starts = np.searchsorted(run_key, np.arange(C * NSP * NB))
    ends = np.append(starts[1:], len(run_key))

    # unified layout: per half, concatenated padded (h, db) runs; each half's
    # total padded up to a chunk (128) multiple
    off_hb = np.zeros((NSP, NB), np.int64)
    POS = [0] * NSP
    for h in range(NSP):
        p = 0
        for b in range(NB):
            off_hb[h, b] = p
            p += int(Lhb[h, b])
        POS[h] = -(-p // 128) * 128
    NCHUNK = [p // 128 for p in POS]

    # pieces: (chunk, db, s0, s1) — intersection of a 128-chunk with a run.
    # Emitted in position order (runs are disjoint ascending intervals).
    pieces = [[] for _ in range(NSP)]
    for h in range(NSP):
        for b in range(NB):
            lo, hi = int(off_hb[h, b]), int(off_hb[h, b] + Lhb[h, b])
            if lo == hi:
                continue
            for cc in range(lo // 128, (hi - 1) // 128 + 1):
                s0 = max(lo, cc * 128) - cc * 128
                s1 = min(hi, (cc + 1) * 128) - cc * 128
                pieces[h].append((cc, b, s0, s1))
        # position-ordered check (required for contiguous S streaming)
        ppos = [cc * 128 + s0 for (cc, b, s0, s1) in pieces[h]]
        assert all(ppos[i] < ppos[i + 1] for i in range(len(ppos) - 1))
    NPIECE = [len(p) for p in pieces]
    SW = sum(NPIECE) * 128
    WT = sum(POS) // 16

    # gather calls per split: groups of <=8 chunks
    calls = [[] for _ in range(NSP)]
    for h in range(NSP):
        cc = 0
        while cc < NCHUNK[h]:
            n = min(8, NCHUNK[h] - cc)
            calls[h].append((cc, n))
            cc += n
    # max pieces per call (St tile sizing)
    maxpc = 1
    for h in range(NSP):
        cp = {}
        for pi, (cc, b, s0, s1) in enumerate(pieces[h]):
            cp.setdefault(cc // 8, []).append(pi)
        maxpc = max([maxpc] + [len(v) for v in cp.values()])

    in_maps = []
    for ci in range(C):
        idx_all = np.zeros((128, WT), np.int16)
        s_all = np.zeros((128, SW), BF16)
        for h in range(NSP):
            base = sum(POS[:h]) // 16
            for b in range(NB):
                k = (ci * NSP + h) * NB + b
                a, e = starts[k], ends[k]
                ne = e - a
                if ne == 0:
                    continue
                j = int(off_hb[h, b]) + np.arange(ne)
                t16 = g_tbl[a:e].astype(np.int16)
                cols = base + j // 16
                rows = j % 16
                for grp in range(8):
                    idx_all[grp * 16 + rows, cols] = t16
        poff = 0
        for h in range(NSP):
            for (cc, b, s0, s1) in pieces[h]:
                k = (ci * NSP + h) * NB + b
                a, e = starts[k], ends[k]
                ne = e - a
                glo = cc * 128 + s0 - int(off_hb[h, b])
                ghi = cc * 128 + s1 - int(off_hb[h, b])
                lo, hi = max(0, glo), min(ne, ghi)
                if hi > lo:
                    jj = np.arange(lo, hi)
                    prow = (int(off_hb[h, b]) + jj) % 128
                    s_all[prow, poff + g_dcol[a + lo:a + hi]] = \
                        g_norm[a + lo:a + hi].astype(BF16)
                poff += 128
        xT = np.zeros((256, NPAD), BF16)
        xT[:, :NL] = x[ci * NL:(ci + 1) * NL].T.astype(BF16)
        in_maps.append(dict(
            xT=xT,
            idx_all=idx_all,
            s_all=s_all,
            wi=W_init.reshape(L, 2, 128, F).astype(BF16),
            wr=W_root.reshape(L, 2, 128, F).astype(BF16),
            bias_c=np.ascontiguousarray(
                bias.reshape(L * 2, 128).T.astype(np.float32)),  # [128, L*2]
        ))

    meta = dict(pieces=pieces, calls=calls, NCHUNK=NCHUNK, NPIECE=NPIECE,
                POS=POS, WT=WT, SW=SW, maxpc=maxpc)
    return meta, in_maps


def _nb_split(nb):
    c0 = nb * 128
    for sp_ in range(NSP - 1, -1, -1):
        if c0 >= SOFF[sp_]:
            return sp_


def _h_block(nc, psp, wp, l, nb, xsrc, wi_sb, hb):
    """Emit h = x @ Wi for one node block of layer l into its bounce buffer."""
    bf = mybir.dt.bfloat16
    f32 = mybir.dt.float32
    w = _blkw(nb)
    c0 = nb * 128
    ph = psp.tile([128, F], f32, tag="ph", bufs=3, name=f"ph{l}_{nb}")
    for g in range(2):
        nc.tensor.matmul(out=ph[:w, :], lhsT=xsrc[g][:, c0:c0 + w],
                         rhs=wi_sb[l][g][:], start=(g == 0), stop=(g == 1))
    hsb = wp.tile([128, F], bf, tag="hsb", bufs=6, name=f"hsb{l}_{nb}")
    nc.vector.tensor_copy(hsb[:w, :], ph[:w, :])
    sp_ = _nb_split(nb)
    r0 = c0 - SOFF[sp_]
    nc.sync.dma_start(out=hb[sp_][r0:r0 + w, :], in_=hsb[:w, :])


def _epilogue(nc, psp, wp, t, b, l, ident, bias_sb, xw_, outT, nxt):
    bf = mybir.dt.bfloat16
    f32 = mybir.dt.float32
    wd = _blkw(b)
    c0 = b * 128
    cp = wp.tile([128, F], bf, tag="cp", bufs=3, name=f"cp{l}_{b}")
    nc.vector.tensor_copy(cp[:], t[:])
    for g in range(2):
        pt = psp.tile([128, 128], bf, tag="pt", bufs=2, name=f"pt{l}_{b}_{g}")
        nc.tensor.transpose(out=pt[:], in_=cp[:, g * 128:(g + 1) * 128],
                            identity=ident[:])
        bcol = bias_sb[l * 2 + g][:]
        if l < L - 1:
            nc.scalar.activation(
                out=xw_[g][:, c0:c0 + wd], in_=pt[:, :wd],
                func=mybir.ActivationFunctionType.Relu, bias=bcol)
        else:
            ot = wp.tile([128, 128], f32, tag="ot", bufs=2, name=f"ot{l}_{b}_{g}")
            nc.scalar.activation(
                out=ot[:, :wd], in_=pt[:, :wd],
                func=mybir.ActivationFunctionType.Relu, bias=bcol)
            nc.sync.dma_start(out=outT[g][:, c0:c0 + wd], in_=ot[:, :wd])
    if l < L - 1:
        # queue next layer's h for this block; emitted with a lag so the PE
        # doesn't stall waiting on this epilogue's Act write
        nxt["pend"].append(b)
        _flush_h(nc, psp, wp, l, xw_, nxt, lag=4)


def _flush_h(nc, psp, wp, l, xw_, nxt, lag):
    while len(nxt["pend"]) > lag:
        b = nxt["pend"].pop(0)
        _h_block(nc, psp, wp, l + 1, b, xw_, nxt["wi_sb"], nxt["hb"])
        sp_ = _nb_split(b)
        nxt["rem"][sp_] -= 1
        if nxt["rem"][sp_] == 0:
            nc.gpsimd.collective_compute(
                "AllGather", mybir.AluOpType.bypass,
                replica_groups=nxt["groups"], ins=[nxt["hb"][sp_][:]],
                outs=[nxt["hg"][sp_][:]])


def _build(meta):
    pieces, calls = meta["pieces"], meta["calls"]
    POS, WT, SW = meta["POS"], meta["WT"], meta["SW"]
    NPIECE, maxpc = meta["NPIECE"], meta["maxpc"]
    bf = mybir.dt.bfloat16
    f32 = mybir.dt.float32

    nc = bacc.Bacc("TRN2", target_bir_lowering=False, debug=False, num_devices=C,
                   num_swdge_queues=4, dynamic_dma_scratch_size=32768)
    xT_p = nc.dram_tensor("xT", [256, NPAD], bf, kind="ExternalInput")
    idx_p = nc.dram_tensor("idx_all", [128, WT], mybir.dt.int16, kind="ExternalInput")
    s_p = nc.dram_tensor("s_all", [128, SW], bf, kind="ExternalInput")
    wi_p = nc.dram_tensor("wi", [L, 2, 128, F], bf, kind="ExternalInput")
    wr_p = nc.dram_tensor("wr", [L, 2, 128, F], bf, kind="ExternalInput")
    bias_p = nc.dram_tensor("bias_c", [128, L * 2], f32, kind="ExternalInput")
    outT = [nc.dram_tensor(f"outT{g}", [128, NL], f32, kind="ExternalOutput")
            for g in range(2)]

    groups = [list(range(C))]
    # chunk -> [(local_pi, cc, b, s0, s1)] and db -> last local_pi, per half
    chunk_pieces = [{} for _ in range(NSP)]
    db_last = [[None] * NB for _ in range(NSP)]
    db_any = [[False] * NB for _ in range(NSP)]
    for h in range(NSP):
        for pi, (cc, b, s0, s1) in enumerate(pieces[h]):
            chunk_pieces[h].setdefault(cc, []).append((pi, cc, b))
            db_last[h][b] = pi
            db_any[h][b] = True

    with tile.TileContext(nc) as tc:
        with (
            tc.tile_pool(name="persist", bufs=1) as pp,
            tc.tile_pool(name="dram", bufs=2, space="DRAM") as dp,
            tc.tile_pool(name="psum", bufs=3, space="PSUM") as psp,
            tc.tile_pool(name="work", bufs=3) as wp,
        ):
            ident = pp.tile([128, 128], bf)
            make_identity(nc, ident[:])
            idx_sb = pp.tile([128, WT], mybir.dt.int16)
            nc.sync.dma_start(out=idx_sb[:], in_=idx_p[:])
            bias_sb = [pp.tile([128, 1], f32, name=f"bias{c}") for c in range(L * 2)]
            for c_ in range(L * 2):
                nc.sync.dma_start(out=bias_sb[c_][:], in_=bias_p[:, c_:c_ + 1])
            wi_sb = [[pp.tile([128, F], bf, name=f"wi{l}{g}") for g in range(2)]
                     for l in range(L)]
            wr_sb = [[pp.tile([128, F], bf, name=f"wr{l}{g}") for g in range(2)]
                     for l in range(L)]
            for l in range(L):
                for g in range(2):
                    nc.sync.dma_start(out=wi_sb[l][g][:], in_=wi_p[l, g])
                    nc.sync.dma_start(out=wr_sb[l][g][:], in_=wr_p[l, g])
            xa = [pp.tile([128, NPAD], bf, name=f"xa{g}") for g in range(2)]
            xb = [pp.tile([128, NPAD], bf, name=f"xb{g}") for g in range(2)]
            for g in range(2):
                nc.sync.dma_start(out=xa[g][:], in_=xT_p[g * 128:(g + 1) * 128, :])
                if NPAD > NL:
                    nc.gpsimd.memset(xb[g][:, NL:], 0.0)
            acc_sb = pp.tile([128, NB * F], bf)   # pass A -> pass B spill

            # per-layer bounce/gather-table tiles (bufs=2 ping-pong)
            hb = []
            hg = []
            for l in range(L):
                hb.append(tuple(dp.tile([SPLITS[sp_], F], bf, tag=f"hb{sp_}",
                                        name=f"hb{sp_}_{l}")
                                for sp_ in range(NSP)))
                hg.append(tuple(dp.tile([TBL[sp_], F], bf, addr_space="Shared",
                                        tag=f"hg{sp_}", name=f"hg{sp_}_{l}")
                                for sp_ in range(NSP)))

            # prologue: layer 0 h-phase + collectives
            nbounds = [SOFF[i] // 128 for i in range(1, NSP)] + [NB]
            for nb in range(NB):
                _h_block(nc, psp, wp, 0, nb, xa, wi_sb, hb[0])
                for sp_ in range(NSP):
                    if nb == nbounds[sp_] - 1:
                        nc.gpsimd.collective_compute(
                            "AllGather", mybir.AluOpType.bypass,
                            replica_groups=groups, ins=[hb[0][sp_][:]],
                            outs=[hg[0][sp_][:]])

            qn = [0]
            for l in range(L):
                xr_ = xa if l % 2 == 0 else xb
                xw_ = xb if l % 2 == 0 else xa
                if l < L - 1:
                    nxt = dict(wi_sb=wi_sb, hb=hb[l + 1], hg=hg[l + 1],
                               rem=[(SOFF + [NL])[i + 1] // 128 - SOFF[i] // 128
                                    if i < NSP - 1 else NB - SOFF[i] // 128
                                    for i in range(NSP)],
                               groups=groups, pend=[])
                else:
                    nxt = None

                # ---- message passing: pass A then pass B ----
                Gr = pp.tile([128, GRING, F], bf, name=f"Gr{l}", tag="Gr")
                pa = {}
                for h in range(NSP):
                    hgx = hg[l][h]
                    ibase = sum(POS[:h]) // 16
                    pbase = sum(NPIECE[:h])
                    for (clo, ncnk) in calls[h]:
                        slot0 = clo % GRING
                        nidx = ncnk * 128
                        nc.gpsimd.dma_gather(
                            out_ap=Gr[:, slot0:slot0 + ncnk, :], in_ap=hgx[:],
                            idxs_ap=idx_sb[:, ibase + clo * 8:
                                           ibase + (clo + ncnk) * 8],
                            num_idxs=nidx, num_idxs_reg=nidx,
                            elem_size=F, queue_num=qn[0] % 4)
                        qn[0] += 1
                        plist = []
                        for cc in range(clo, clo + ncnk):
                            plist += chunk_pieces[h].get(cc, [])
                        if not plist:
                            continue
                        p0 = plist[0][0]
                        np_ = len(plist)
                        St = wp.tile([128, maxpc * 128], bf, tag="St", bufs=4,
                                     name=f"St{l}_{h}_{clo}")
                        nc.sync.dma_start(
                            out=St[:, :np_ * 128],
                            in_=s_p[:, (pbase + p0) * 128:(pbase + p0 + np_) * 128])
                        for k, (pi, cc, b) in enumerate(plist):
                            if b not in pa:
                                t = psp.tile([128, F], f32, tag="pa", bufs=3,
                                             name=f"pa{l}_{h}_{b}")
                                pa[b] = t
                                if any(db_any[hh][b] for hh in range(h)):
                                    nc.tensor.matmul(
                                        out=t[:], lhsT=ident[:],
                                        rhs=acc_sb[:, b * F:(b + 1) * F],
                                        start=True, stop=False)
                                else:
                                    for g in range(2):
                                        nc.tensor.matmul(
                                            out=t[:],
                                            lhsT=xr_[g][:, b * 128:b * 128 + 128],
                                            rhs=wr_sb[l][g][:],
                                            start=(g == 0), stop=False)
                            last_piece = (pi == db_last[h][b])
                            nc.tensor.matmul(
                                out=pa[b][:],
                                lhsT=St[:, k * 128:(k + 1) * 128],
                                rhs=Gr[:, slot0 + (cc - clo), :],
                                start=False, stop=last_piece)
                            if last_piece:
                                t = pa.pop(b)
                                if any(db_any[hh][b] for hh in range(h + 1, NSP)):
                                    nc.vector.tensor_copy(
                                        acc_sb[:, b * F:(b + 1) * F], t[:])
                                else:
                                    _epilogue(nc, psp, wp, t, b, l, ident,
                                              bias_sb, xw_, outT, nxt)
                assert not pa
                if nxt is not None:
                    _flush_h(nc, psp, wp, l, xw_, nxt, lag=0)
                # dbs with no edges at all (xr + bias + relu only)
                for b in range(NB):
                    if not any(db_any[hh][b] for hh in range(NSP)):
                        t = psp.tile([128, F], f32, tag="pa", bufs=3,
                                     name=f"paz{l}_{b}")
                        for g in range(2):
                            nc.tensor.matmul(
                                out=t[:], lhsT=xr_[g][:, b * 128:b * 128 + 128],
                                rhs=wr_sb[l][g][:], start=(g == 0), stop=(g == 1))
                        _epilogue(nc, psp, wp, t, b, l, ident, bias_sb,
                                  xw_, outT, nxt)
    nc.compile()
    return nc


_CACHE = {}


def kernel(**inputs):
    meta, in_maps = _preprocess(**inputs)
    key = (tuple(map(tuple, meta["calls"][0])), tuple(map(tuple, meta["calls"][1])),
           tuple(map(tuple, meta["pieces"][0])), tuple(map(tuple, meta["pieces"][1])))
    nc = _CACHE.get(key)
    if nc is None:
        nc = _build(meta)
        _CACHE[key] = nc
    res = run_bass_kernel_spmd(nc, in_maps, list(range(C)), trace=False)
    out = np.empty((N, F), np.float32)
    for ci in range(C):
        r = res.results[ci]
        xt = np.concatenate([r["outT0"], r["outT1"]], axis=0)  # [256, NL]
        out[ci * NL:(ci + 1) * NL] = xt.T
    return out
